# revision 26
# baseline (speedup 1.0000x reference)
"""Fused cross-entropy label-propagation kernel for Trainium2 (8 cores), v2.

Computation (per batch b):
  sim   = ref_flat(b) @ tgt_flat(b)          # [12288, 4096]
  prob  = softmax(sim, axis=0)               # over ref pixels
  pred  = lab_flat(b) @ prob                 # [16, 4096]
  loss  = mean(-log(pred[label] + eps))

Sharding: batch b = core // 4, target-pixel columns split 4-way per batch
(softmax is over the ref axis, so column sharding needs no communication).

v2 changes over the ACT-bound v1 (98.6us -> ~65us):
1. The exp of all 12.6M sim values per core was the bottleneck (ScalarE is
   the only exp engine, 1 elem/cycle/lane @1.2GHz = 82us floor). The exp
   is now SPLIT between ScalarE (exact exp, ~52% of slots) and the DVE
   using Schraudolph's trick: the host prescales ref/tgt by
   sqrt(128*log2 e) so the PE produces sim' = 128*log2e*s in PSUM; the
   DVE then computes int16(max(sim' + B_ADD, 0)) in one tensor_scalar op
   (round-to-nearest, saturating), whose int16 bit pattern reinterpreted
   as bf16 equals e^(s-shift) within +-3%. The pipeline already tolerates
   +-49% per-element noise from fp8 (1.6e-3 final loss error), so +-3% on
   half the tiles is noise (measured end-to-end: 3.7e-4 rel error).
   ScalarE applies exp((1/A_SCALE)*sim' - shift) via its free affine, so
   both engines read the same PSUM tiles.
2. The label matmul (M=17, 13% PE array use) is packed 4x with col-tiling
   (tile_position=(0,32j)): 4 k-tiles' matmuls run concurrently in four
   32-column groups of the PE array (HW-measured 5.7x), accumulating into
   four partition-slices of a [128, 1024] PSUM tile. The host sums the 4
   slices. 41us -> ~7us of PE time.
3. Exp slots are 512 wide (1 PSUM bank) in a 6-deep ring: a deeper ring
   was HW-measured faster than wider-instruction/shallower-ring layouts
   (the exp(k) -> fill(k+ring) -> exp(k+ring) WAR chain gets 6 slots of
   slack, hiding PE-fill and semaphore latency).

PSUM: 6 sim slots (1 bank) + pred (2 banks) = 8 banks exactly.

The constant shift replaces the per-column max (data-dependent rescue on
the host reruns with shift +-60/120 if any column's exp window overflowed;
never triggered on reference-like data, where col maxima are 57..220).
Schraudolph saturation at s-shift > 97.6 would produce NaN/garbage in the
affected column; the same host check catches that case too.

Host finishes with slice-combine, num/den, log, gather, mean (float64).
"""

import math
import os

import numpy as np
import ml_dtypes

B, NREF, F, H, W, D = 2, 3, 256, 64, 64, 16
T = H * W                     # 4096 target pixels per batch
N = NREF * T                  # 12288 ref pixels per batch
NCORES = 8
T_LOC = B * T // NCORES       # 1024 columns per core
NT = N // 128                 # 96 ref-row tiles (one exp slot each)
NSLOT = NT                    # exp slots: [128, 1024], 2 PSUM banks
NPACK = NT // 4               # 24 col-tiled label packs
NCHUNK = 8                    # ref DMA chunks (12 k-tiles each)
KPC = NT // NCHUNK            # k-tiles per chunk
SHIFT0 = 138.5                # subtracted from sim before exp (host-adjustable)
EPS = 1e-14
LOG2E = math.log2(math.e)
A_SCALE = 128.0 * LOG2E       # PE computes sim' = A_SCALE * s
SQA = math.sqrt(A_SCALE)      # host folds sqrt into each fp8 operand
SIGMA = 5.5104                # Schraudolph bias: min-max relative error

LAG = int(os.environ.get("KLAG", "4"))          # label packs lag, in slots
ACT_SHARE = float(os.environ.get("KACT", "0.52"))   # exp share on ScalarE
SLOTW = int(os.environ.get("KSLOTW", "512"))   # exp slot width (512 or 1024)
SIMBUFS = {512: 6, 1024: 3}[SLOTW]              # PSUM ring depth
PBUFS = int(os.environ.get("KPBUFS",
                           str(LAG + {512: 12, 1024: 7}[SLOTW])))

FP8 = ml_dtypes.float8_e4m3
BF16 = ml_dtypes.bfloat16

_CACHE = {}
LAST_RESULTS = None  # BassKernelResults of the most recent run (for profiling)


GROUP = int(os.environ.get("KGROUP", "1"))      # engine-assignment run length


def _exp_schedule(nslots):
    """Per-slot engine assignment: 'A' (ScalarE exact exp) or 'V' (DVE
    Schraudolph). Interleaved in runs of GROUP to keep both engines fed."""
    sched = []
    cum = 0.0
    for _ in range((nslots + GROUP - 1) // GROUP):
        cum += ACT_SHARE
        if cum >= 1.0:
            sched.extend("A" * GROUP)
            cum -= 1.0
        else:
            sched.extend("V" * GROUP)
    return sched[:nslots]


def _build_program(reps=1, shift=SHIFT0):
    # reps > 1 repeats the whole compute body (timing harness only; the extra
    # reps recompute the same result into the same output).
    key = ("nc", reps, shift, LAG, ACT_SHARE, PBUFS, SLOTW)
    if key in _CACHE:
        return _CACHE[key]

    import concourse.bacc as bacc
    import concourse.tile as tile
    import concourse.mybir as mybir

    f32 = mybir.dt.float32
    bf16 = mybir.dt.bfloat16
    i16 = mybir.dt.int16
    fp8 = mybir.dt.float8e4

    b_add = 128.0 * (127.0 - LOG2E * shift) - SIGMA
    upslot = SLOTW // 512            # 512-col units per slot (1 or 2)
    nslots = 2 * NT // upslot
    sched = _exp_schedule(nslots)

    nc = bacc.Bacc("TRN2", target_bir_lowering=False, debug=False,
                   num_devices=NCORES)

    # Per-core inputs, pre-laid-out on host so every DMA is contiguous.
    ref_d = nc.dram_tensor("ref", [NCHUNK, 128, KPC, 2, 128], fp8,
                           kind="ExternalInput")
    tgt_d = nc.dram_tensor("tgt", [128, 2, T_LOC], fp8, kind="ExternalInput")
    lab_d = nc.dram_tensor("lab", [128, NT, D + 1], bf16,
                           kind="ExternalInput")
    out_d = nc.dram_tensor("out", [128, T_LOC], f32, kind="ExternalOutput")

    with tile.TileContext(nc) as tc:
        with (
            tc.tile_pool(name="small", bufs=1) as small,
            tc.tile_pool(name="ppool", bufs=PBUFS) as ppool,
            tc.tile_pool(name="simpool", bufs=SIMBUFS, space="PSUM") as simpool,
            tc.tile_pool(name="predpool", bufs=1, space="PSUM") as predpool,
        ):
            # Warm the ScalarE exp table immediately (the ~2.7us
            # ACT_TABLE_LOAD runs under the input DMAs instead of on the
            # critical path of the first real exp).
            po0 = small.tile([128, 512], f32, tag="po0")
            po1 = small.tile([128, 512], f32, tag="po1")
            dummy = small.tile([128, 1], f32, tag="dummy")
            nc.scalar.activation(out=dummy, in_=po0[:, 0:1],
                                 func=mybir.ActivationFunctionType.Exp,
                                 scale=1.0)

            # Startup-critical loads split across issue queues so the first
            # sim matmul + exp only wait on tiny transfers.
            tgt_sb = small.tile([128, 2, T_LOC], fp8, tag="tgt")
            nc.sync.dma_start(out=tgt_sb[:, :, 0:512], in_=tgt_d[:, :, 0:512])
            ref_sb = small.tile([128, NT, 2, 128], fp8, tag="ref")
            nc.gpsimd.dma_start(out=ref_sb[:, 0:2], in_=ref_d[0][:, 0:2])
            bias_sb = small.tile([128, 1], f32, tag="bias")
            nc.gpsimd.memset(bias_sb, -shift)
            lab_sb = small.tile([128, NT, D + 1], bf16, tag="lab")
            nc.gpsimd.dma_start(out=lab_sb[:, 0:8], in_=lab_d[:, 0:8])
            nc.sync.dma_start(out=tgt_sb[:, :, 512:], in_=tgt_d[:, :, 512:])
            nc.sync.dma_start(out=ref_sb[:, 2:5], in_=ref_d[0][:, 2:5])
            nc.sync.dma_start(out=ref_sb[:, 5:KPC], in_=ref_d[0][:, 5:KPC])
            nc.sync.dma_start(out=lab_sb[:, 8:], in_=lab_d[:, 8:])
            for c in range(1, NCHUNK):
                nc.sync.dma_start(out=ref_sb[:, c * KPC:(c + 1) * KPC],
                                  in_=ref_d[c])

            # Label pack q covers k-tiles 4q..4q+3 (p slots 4q..4q+3, ready
            # at slot 4q+3); emitted LAG slots after that. The four j's run
            # concurrently in distinct 32-column groups of the PE array.
            def label_half_pack(q, h, slot_p, pred):
                for j in range(4):
                    k = 4 * q + j
                    u = 2 * k + h
                    rhs = (slot_p[k][:, h * 512:(h + 1) * 512]
                           if upslot == 2 else slot_p[u])
                    nc.tensor.matmul(
                        pred[32 * j:32 * j + 17, h * 512:(h + 1) * 512],
                        lhsT=lab_sb[:, k],
                        rhs=rhs,
                        start=(q == 0), stop=(q == NPACK - 1),
                        tile_position=(0, 32 * j),
                    )

            def drain(pred):
                nc.vector.tensor_copy(po0, pred[:, :512])
                nc.scalar.copy(po1, pred[:, 512:])
                if not os.environ.get("KNODMA"):
                    nc.sync.dma_start(out=out_d[:, :512], in_=po0)
                    nc.gpsimd.dma_start(out=out_d[:, 512:], in_=po1)

            for rep in range(reps):
                pred = predpool.tile([128, T_LOC], f32, tag="pred")
                slot_p = {}
                nextq = 0
                for s in range(nslots):
                    sim = simpool.tile([128, SLOTW], f32, tag="sim")
                    for i in range(upslot):
                        u = s * upslot + i
                        k, h = divmod(u, 2)
                        nc.tensor.matmul(
                            sim[:, 512 * i:512 * (i + 1)],
                            lhsT=ref_sb[:, k],
                            rhs=tgt_sb[:, :, 512 * h:512 * (h + 1)],
                            start=True, stop=True,
                            perf_mode=mybir.MatmulPerfMode.DoubleRow,
                        )
                    p = ppool.tile([128, SLOTW], bf16, tag="p")
                    if sched[s] == "A":
                        nc.scalar.activation(
                            out=p, in_=sim,
                            func=mybir.ActivationFunctionType.Exp,
                            bias=bias_sb[:], scale=1.0 / A_SCALE)
                    else:
                        nc.vector.tensor_scalar(
                            out=p.bitcast(i16), in0=sim,
                            scalar1=b_add, scalar2=0.0,
                            op0=mybir.AluOpType.add,
                            op1=mybir.AluOpType.max)
                    slot_p[s] = p
                    # pack q ready once its last unit's slot is done
                    while (nextq < NPACK
                           and (8 * nextq + 7) // upslot + LAG <= s):
                        for h in range(2):
                            label_half_pack(nextq, h, slot_p, pred)
                        nextq += 1
                while nextq < NPACK:
                    for h in range(2):
                        label_half_pack(nextq, h, slot_p, pred)
                    nextq += 1
                drain(pred)

    nc.compile()
    _CACHE[key] = nc
    return nc


def _prep_inputs(ref, target, ref_label):
    """Per-batch host-side relayouts shared by the 4 cores of each batch.
    ref/target are prescaled by sqrt(128*log2 e) so the PE's sim output is
    already in Schraudolph exponent units."""
    per_b = []
    for b in range(B):
        # ref tile layout for DoubleRow: [chunk, f_lo(part), k_in_chunk,
        # j(f_hi), n_in_tile], fp8e4m3
        rf = (ref[b] * SQA).astype(FP8)              # [3, 256, 64, 64]
        rf = rf.reshape(NREF, 2, 128, T)             # [r, j, f_lo, hw]
        rf = rf.transpose(0, 3, 1, 2)                # [r, hw, j, f_lo]
        rf = rf.reshape(NT, 128, 2, 128)             # [k, nn, j, f_lo]
        rf = rf.transpose(0, 3, 2, 1)                # [k, f_lo, j, nn]
        rf = rf.reshape(NCHUNK, KPC, 128, 2, 128)
        refb = np.ascontiguousarray(rf.transpose(0, 2, 1, 3, 4))
        # target: [f_lo(part), j, t], fp8
        tg = (target[b] * SQA).astype(FP8).reshape(2, 128, T)
        tgtb = np.ascontiguousarray(tg.transpose(1, 0, 2))
        # labels: n = (r, h, w) major -> [12288, 16], append ones -> [.., 17]
        labn = ref_label[b].transpose(0, 2, 3, 1).reshape(N, D)
        labo = np.concatenate(
            [labn, np.ones((N, 1), np.float32)], axis=1)
        # -> SBUF layout [128(part), 96, 17]: sb[p, k, j] = labo[k*128+p, j]
        labsb = np.ascontiguousarray(
            labo.reshape(NT, 128, D + 1).transpose(1, 0, 2)).astype(BF16)
        per_b.append((refb, labsb, tgtb))
    return per_b


def _run_cores(per_b, shift):
    """One SPMD run with the given softmax shift; returns per-batch
    [17, 4096] float64 (the four col-tiled partition slices summed)."""
    global LAST_RESULTS
    from concourse.bass_utils import run_bass_kernel_spmd

    nc = _build_program(shift=shift)
    in_maps = []
    for core in range(NCORES):
        b, s = divmod(core, NCORES // B)
        refb, labsb, tgtb = per_b[b]
        in_maps.append({
            "ref": refb,
            "tgt": np.ascontiguousarray(tgtb[:, :, s * T_LOC:(s + 1) * T_LOC]),
            "lab": labsb,
        })
    LAST_RESULTS = run_bass_kernel_spmd(nc, in_maps, list(range(NCORES)))
    outs = LAST_RESULTS.results
    res = []
    for b in range(B):
        cols = []
        for s in range(NCORES // B):
            raw = outs[b * (NCORES // B) + s]["out"].astype(np.float64)
            comb = sum(raw[32 * j:32 * j + D + 1] for j in range(4))
            cols.append(comb)
        res.append(np.concatenate(cols, axis=1))
    return res


def _bad_cols(raw):
    """Columns whose exp window overflowed/underflowed for the used shift."""
    with np.errstate(all="ignore"):
        den, num = raw[D], raw[:D]
        return ~np.isfinite(den) | (den <= 0.0) | ~np.isfinite(num).all(axis=0)


def kernel(ref, target, ref_label, target_label):
    ref = np.asarray(ref, np.float32)
    target = np.asarray(target, np.float32)
    ref_label = np.asarray(ref_label, np.float32)
    labels = np.asarray(target_label).astype(np.int64)

    per_b = _prep_inputs(ref, target, ref_label)
    raws = _run_cores(per_b, SHIFT0)

    # Rescue any columns outside the exp window with shifted reruns (a no-op
    # for data resembling the reference distribution).
    bad = [_bad_cols(r) for r in raws]
    for delta in (60.0, -60.0, 120.0, -120.0):
        if not any(bm.any() for bm in bad):
            break
        raws2 = _run_cores(per_b, SHIFT0 + delta)
        for b in range(B):
            fixable = bad[b] & ~_bad_cols(raws2[b])
            raws[b][:, fixable] = raws2[b][:, fixable]
            bad[b] &= ~fixable

    nll_sum = 0.0
    with np.errstate(all="ignore"):
        for b in range(B):
            pred = raws[b][:D] / raws[b][D]                  # [16, 4096]
            logp = np.log(pred + EPS)
            idx = labels[b].reshape(T)
            nll_sum += -logp[idx, np.arange(T)].sum()
    loss = nll_sum / (B * T)
    return np.asarray(loss, dtype=np.float32)


# revision 27
# speedup vs baseline: 1.1137x; 1.1137x over previous
"""Fused cross-entropy label-propagation kernel for Trainium2 (8 cores), v2.

Computation (per batch b):
  sim   = ref_flat(b) @ tgt_flat(b)          # [12288, 4096]
  prob  = softmax(sim, axis=0)               # over ref pixels
  pred  = lab_flat(b) @ prob                 # [16, 4096]
  loss  = mean(-log(pred[label] + eps))

Sharding: batch b = core // 4, target-pixel columns split 4-way per batch
(softmax is over the ref axis, so column sharding needs no communication).

v2 changes over the ACT-bound v1 (98.6us -> ~65us):
1. The exp of all 12.6M sim values per core was the bottleneck (ScalarE is
   the only exp engine, 1 elem/cycle/lane @1.2GHz = 82us floor). The exp
   is now SPLIT between ScalarE (exact exp, ~52% of slots) and the DVE
   using Schraudolph's trick: the host prescales ref/tgt by
   sqrt(128*log2 e) so the PE produces sim' = 128*log2e*s in PSUM; the
   DVE then computes int16(max(sim' + B_ADD, 0)) in one tensor_scalar op
   (round-to-nearest, saturating), whose int16 bit pattern reinterpreted
   as bf16 equals e^(s-shift) within +-3%. The pipeline already tolerates
   +-49% per-element noise from fp8 (1.6e-3 final loss error), so +-3% on
   half the tiles is noise (measured end-to-end: 3.7e-4 rel error).
   ScalarE applies exp((1/A_SCALE)*sim' - shift) via its free affine, so
   both engines read the same PSUM tiles.
2. The label matmul (M=17, 13% PE array use) is packed 4x with col-tiling
   (tile_position=(0,32j)): 4 k-tiles' matmuls run concurrently in four
   32-column groups of the PE array (HW-measured 5.7x), accumulating into
   four partition-slices of a [128, 1024] PSUM tile. The host sums the 4
   slices. 41us -> ~7us of PE time.
3. Exp slots are 512 wide (1 PSUM bank) in a 6-deep ring: a deeper ring
   was HW-measured faster than wider-instruction/shallower-ring layouts
   (the exp(k) -> fill(k+ring) -> exp(k+ring) WAR chain gets 6 slots of
   slack, hiding PE-fill and semaphore latency).

PSUM: 6 sim slots (1 bank) + pred (2 banks) = 8 banks exactly.

The constant shift replaces the per-column max (data-dependent rescue on
the host reruns with shift +-60/120 if any column's exp window overflowed;
never triggered on reference-like data, where col maxima are 57..220).
Schraudolph saturation at s-shift > 97.6 would produce NaN/garbage in the
affected column; the same host check catches that case too.

Host finishes with slice-combine, num/den, log, gather, mean (float64).
"""

import math
import os

import numpy as np
import ml_dtypes

B, NREF, F, H, W, D = 2, 3, 256, 64, 64, 16
T = H * W                     # 4096 target pixels per batch
N = NREF * T                  # 12288 ref pixels per batch
NCORES = 8
T_LOC = B * T // NCORES       # 1024 columns per core
NT = N // 128                 # 96 ref-row tiles (one exp slot each)
NSLOT = NT                    # exp slots: [128, 1024], 2 PSUM banks
NPACK = NT // 4               # 24 col-tiled label packs
NCHUNK = 8                    # ref DMA chunks (12 k-tiles each)
KPC = NT // NCHUNK            # k-tiles per chunk
SHIFT0 = 138.5                # subtracted from sim before exp (host-adjustable)
EPS = 1e-14
LOG2E = math.log2(math.e)
A_SCALE = 128.0 * LOG2E       # PE computes sim' = A_SCALE * s
SQA = math.sqrt(A_SCALE)      # host folds sqrt into each fp8 operand
SIGMA = 5.5104                # Schraudolph bias: min-max relative error

LAG = int(os.environ.get("KLAG", "4"))          # label packs lag, in slots
ACT_SHARE = float(os.environ.get("KACT", "0.52"))   # exp share on ScalarE
SLOTW = int(os.environ.get("KSLOTW", "512"))   # exp slot width (512 or 1024)
SIMBUFS = {512: 6, 1024: 3}[SLOTW]              # PSUM ring depth
PBUFS = int(os.environ.get("KPBUFS",
                           str(LAG + {512: 12, 1024: 7}[SLOTW])))

FP8 = ml_dtypes.float8_e4m3
BF16 = ml_dtypes.bfloat16

_CACHE = {}
LAST_RESULTS = None  # BassKernelResults of the most recent run (for profiling)


GROUP = int(os.environ.get("KGROUP", "1"))      # engine-assignment run length


def _exp_schedule(nslots):
    """Per-slot engine assignment: 'A' (ScalarE exact exp) or 'V' (DVE
    Schraudolph). Interleaved in runs of GROUP to keep both engines fed."""
    sched = []
    cum = 0.0
    for _ in range((nslots + GROUP - 1) // GROUP):
        cum += ACT_SHARE
        if cum >= 1.0:
            sched.extend("A" * GROUP)
            cum -= 1.0
        else:
            sched.extend("V" * GROUP)
    return sched[:nslots]


def _build_program(reps=1, shift=SHIFT0):
    # reps > 1 repeats the whole compute body (timing harness only; the extra
    # reps recompute the same result into the same output).
    key = ("nc", reps, shift, LAG, ACT_SHARE, PBUFS, SLOTW)
    if key in _CACHE:
        return _CACHE[key]

    import concourse.bacc as bacc
    import concourse.tile as tile
    import concourse.mybir as mybir

    f32 = mybir.dt.float32
    bf16 = mybir.dt.bfloat16
    i16 = mybir.dt.int16
    fp8 = mybir.dt.float8e4

    b_add = 128.0 * (127.0 - LOG2E * shift) - SIGMA
    upslot = SLOTW // 512            # 512-col units per slot (1 or 2)
    nslots = 2 * NT // upslot
    sched = _exp_schedule(nslots)

    nc = bacc.Bacc("TRN2", target_bir_lowering=False, debug=False,
                   num_devices=NCORES)

    # Per-core inputs, pre-laid-out on host so every DMA is contiguous.
    ref_d = nc.dram_tensor("ref", [NCHUNK, 128, KPC, 2, 128], fp8,
                           kind="ExternalInput")
    tgt_d = nc.dram_tensor("tgt", [128, 2, T_LOC], fp8, kind="ExternalInput")
    lab_d = nc.dram_tensor("lab", [128, NT, D + 1], bf16,
                           kind="ExternalInput")
    out_d = nc.dram_tensor("out", [128, T_LOC], f32, kind="ExternalOutput")

    with tile.TileContext(nc) as tc:
        with (
            tc.tile_pool(name="small", bufs=1) as small,
            tc.tile_pool(name="ppool", bufs=PBUFS) as ppool,
            tc.tile_pool(name="simpool", bufs=SIMBUFS, space="PSUM") as simpool,
            tc.tile_pool(name="predpool", bufs=1, space="PSUM") as predpool,
        ):
            # Warm the ScalarE exp table immediately (the ~2.7us
            # ACT_TABLE_LOAD runs under the input DMAs instead of on the
            # critical path of the first real exp).
            po0 = small.tile([128, 512], f32, tag="po0")
            po1 = small.tile([128, 512], f32, tag="po1")
            dummy = small.tile([128, 1], f32, tag="dummy")
            nc.scalar.activation(out=dummy, in_=po0[:, 0:1],
                                 func=mybir.ActivationFunctionType.Exp,
                                 scale=1.0)

            # Startup-critical loads split across issue queues so the first
            # sim matmul + exp only wait on tiny transfers.
            tgt_sb = small.tile([128, 2, T_LOC], fp8, tag="tgt")
            nc.sync.dma_start(out=tgt_sb[:, :, 0:512], in_=tgt_d[:, :, 0:512])
            ref_sb = small.tile([128, NT, 2, 128], fp8, tag="ref")
            nc.gpsimd.dma_start(out=ref_sb[:, 0:2], in_=ref_d[0][:, 0:2])
            bias_sb = small.tile([128, 1], f32, tag="bias")
            nc.gpsimd.memset(bias_sb, -shift)
            lab_sb = small.tile([128, NT, D + 1], bf16, tag="lab")
            nc.gpsimd.dma_start(out=lab_sb[:, 0:8], in_=lab_d[:, 0:8])
            nc.sync.dma_start(out=tgt_sb[:, :, 512:], in_=tgt_d[:, :, 512:])
            nc.sync.dma_start(out=ref_sb[:, 2:5], in_=ref_d[0][:, 2:5])
            nc.sync.dma_start(out=ref_sb[:, 5:KPC], in_=ref_d[0][:, 5:KPC])
            nc.sync.dma_start(out=lab_sb[:, 8:], in_=lab_d[:, 8:])
            for c in range(1, NCHUNK):
                nc.sync.dma_start(out=ref_sb[:, c * KPC:(c + 1) * KPC],
                                  in_=ref_d[c])

            # Label pack q covers k-tiles 4q..4q+3 (p slots 4q..4q+3, ready
            # at slot 4q+3); emitted LAG slots after that. The four j's run
            # concurrently in distinct 32-column groups of the PE array.
            def label_half_pack(q, h, slot_p, pred):
                for j in range(4):
                    k = 4 * q + j
                    u = 2 * k + h
                    rhs = (slot_p[k][:, h * 512:(h + 1) * 512]
                           if upslot == 2 else slot_p[u])
                    nc.tensor.matmul(
                        pred[32 * j:32 * j + 17, h * 512:(h + 1) * 512],
                        lhsT=lab_sb[:, k],
                        rhs=rhs,
                        start=(q == 0), stop=(q == NPACK - 1),
                        tile_position=(0, 32 * j),
                    )

            def drain(pred):
                nc.vector.tensor_copy(po0, pred[:, :512])
                nc.scalar.copy(po1, pred[:, 512:])
                if not os.environ.get("KNODMA"):
                    nc.sync.dma_start(out=out_d[:, :512], in_=po0)
                    nc.gpsimd.dma_start(out=out_d[:, 512:], in_=po1)

            for rep in range(reps):
                pred = predpool.tile([128, T_LOC], f32, tag="pred")
                slot_p = {}
                nextq = 0
                for s in range(nslots):
                    sim = simpool.tile([128, SLOTW], f32, tag="sim")
                    for i in range(upslot):
                        u = s * upslot + i
                        k, h = divmod(u, 2)
                        nc.tensor.matmul(
                            sim[:, 512 * i:512 * (i + 1)],
                            lhsT=ref_sb[:, k],
                            rhs=tgt_sb[:, :, 512 * h:512 * (h + 1)],
                            start=True, stop=True,
                            perf_mode=mybir.MatmulPerfMode.DoubleRow,
                        )
                    p = ppool.tile([128, SLOTW], bf16, tag="p")
                    # Optional: read only the high 16 bits of each f32 PSUM
                    # word (a bf16 truncation of sim', ~0.2% rel) to halve
                    # PSUM read bytes on the exp engines.
                    sim_h = sim.bitcast(bf16)[:, 1::2]
                    if sched[s] == "A":
                        nc.scalar.activation(
                            out=p,
                            in_=sim_h if os.environ.get("KACTBF") else sim,
                            func=mybir.ActivationFunctionType.Exp,
                            bias=bias_sb[:], scale=1.0 / A_SCALE)
                    else:
                        nc.vector.tensor_scalar(
                            out=p.bitcast(i16),
                            in0=sim_h if os.environ.get("KDVEBF") else sim,
                            scalar1=b_add, scalar2=0.0,
                            op0=mybir.AluOpType.add,
                            op1=mybir.AluOpType.max)
                    slot_p[s] = p
                    # pack q ready once its last unit's slot is done
                    while (nextq < NPACK
                           and (8 * nextq + 7) // upslot + LAG <= s):
                        for h in range(2):
                            label_half_pack(nextq, h, slot_p, pred)
                        nextq += 1
                while nextq < NPACK:
                    for h in range(2):
                        label_half_pack(nextq, h, slot_p, pred)
                    nextq += 1
                drain(pred)

    nc.compile()
    _CACHE[key] = nc
    return nc


def _prep_inputs(ref, target, ref_label):
    """Per-batch host-side relayouts shared by the 4 cores of each batch.
    ref/target are prescaled by sqrt(128*log2 e) so the PE's sim output is
    already in Schraudolph exponent units."""
    per_b = []
    for b in range(B):
        # ref tile layout for DoubleRow: [chunk, f_lo(part), k_in_chunk,
        # j(f_hi), n_in_tile], fp8e4m3
        rf = (ref[b] * SQA).astype(FP8)              # [3, 256, 64, 64]
        rf = rf.reshape(NREF, 2, 128, T)             # [r, j, f_lo, hw]
        rf = rf.transpose(0, 3, 1, 2)                # [r, hw, j, f_lo]
        rf = rf.reshape(NT, 128, 2, 128)             # [k, nn, j, f_lo]
        rf = rf.transpose(0, 3, 2, 1)                # [k, f_lo, j, nn]
        rf = rf.reshape(NCHUNK, KPC, 128, 2, 128)
        refb = np.ascontiguousarray(rf.transpose(0, 2, 1, 3, 4))
        # target: [f_lo(part), j, t], fp8
        tg = (target[b] * SQA).astype(FP8).reshape(2, 128, T)
        tgtb = np.ascontiguousarray(tg.transpose(1, 0, 2))
        # labels: n = (r, h, w) major -> [12288, 16], append ones -> [.., 17]
        labn = ref_label[b].transpose(0, 2, 3, 1).reshape(N, D)
        labo = np.concatenate(
            [labn, np.ones((N, 1), np.float32)], axis=1)
        # -> SBUF layout [128(part), 96, 17]: sb[p, k, j] = labo[k*128+p, j]
        labsb = np.ascontiguousarray(
            labo.reshape(NT, 128, D + 1).transpose(1, 0, 2)).astype(BF16)
        per_b.append((refb, labsb, tgtb))
    return per_b


def _run_cores(per_b, shift):
    """One SPMD run with the given softmax shift; returns per-batch
    [17, 4096] float64 (the four col-tiled partition slices summed)."""
    global LAST_RESULTS
    from concourse.bass_utils import run_bass_kernel_spmd

    nc = _build_program(shift=shift)
    in_maps = []
    for core in range(NCORES):
        b, s = divmod(core, NCORES // B)
        refb, labsb, tgtb = per_b[b]
        in_maps.append({
            "ref": refb,
            "tgt": np.ascontiguousarray(tgtb[:, :, s * T_LOC:(s + 1) * T_LOC]),
            "lab": labsb,
        })
    LAST_RESULTS = run_bass_kernel_spmd(nc, in_maps, list(range(NCORES)))
    outs = LAST_RESULTS.results
    res = []
    for b in range(B):
        cols = []
        for s in range(NCORES // B):
            raw = outs[b * (NCORES // B) + s]["out"].astype(np.float64)
            comb = sum(raw[32 * j:32 * j + D + 1] for j in range(4))
            cols.append(comb)
        res.append(np.concatenate(cols, axis=1))
    return res


def _bad_cols(raw):
    """Columns whose exp window overflowed/underflowed for the used shift."""
    with np.errstate(all="ignore"):
        den, num = raw[D], raw[:D]
        return ~np.isfinite(den) | (den <= 0.0) | ~np.isfinite(num).all(axis=0)


def kernel(ref, target, ref_label, target_label):
    ref = np.asarray(ref, np.float32)
    target = np.asarray(target, np.float32)
    ref_label = np.asarray(ref_label, np.float32)
    labels = np.asarray(target_label).astype(np.int64)

    per_b = _prep_inputs(ref, target, ref_label)
    raws = _run_cores(per_b, SHIFT0)

    # Rescue any columns outside the exp window with shifted reruns (a no-op
    # for data resembling the reference distribution).
    bad = [_bad_cols(r) for r in raws]
    for delta in (60.0, -60.0, 120.0, -120.0):
        if not any(bm.any() for bm in bad):
            break
        raws2 = _run_cores(per_b, SHIFT0 + delta)
        for b in range(B):
            fixable = bad[b] & ~_bad_cols(raws2[b])
            raws[b][:, fixable] = raws2[b][:, fixable]
            bad[b] &= ~fixable

    nll_sum = 0.0
    with np.errstate(all="ignore"):
        for b in range(B):
            pred = raws[b][:D] / raws[b][D]                  # [16, 4096]
            logp = np.log(pred + EPS)
            idx = labels[b].reshape(T)
            nll_sum += -logp[idx, np.arange(T)].sum()
    loss = nll_sum / (B * T)
    return np.asarray(loss, dtype=np.float32)
